# revision 1
# baseline (speedup 1.0000x reference)
"""CrossTransformer kernel for Trainium2, data-parallel over batch across 8 cores.

Math per batch b (B=32, N=25, C=512, H=W=14, DK=DV=128):
  qq = Wqk @ Q    [128, 196]      qv = Wv @ Q     [128, 196]
  K  = Wqk @ S    [128, 4900]     V  = Wv @ S     [128, 4900]
  simT[nij, hw] = K^T @ qq        (computed directly in transposed layout)
  E = exp(simT)                   (no max subtraction; |sim| <~ 60 is safe in fp32)
  ctx_raw[hw, v+1] = sum_nij E[nij, hw]^T @ [V^T | 1]   (ones column -> softmax denom)
  ctx = ctx_raw[:, :128] / ctx_raw[:, 128:129]
  partial += sum((qv^T - ctx)^2)
Output per core: scalar partial sum over its 4 batches; host sums and divides by H*W.
"""

import os
import sys

sys.path.insert(0, "/opt/trn_rl_repo")

import numpy as np

import concourse.bass as bass
import concourse.bacc as bacc
import concourse.mybir as mybir
import concourse.tile as tile
from concourse.bass_utils import run_bass_kernel_spmd
from concourse.masks import make_identity

F32 = mybir.dt.float32
F32R = mybir.dt.float32r
BF16 = mybir.dt.bfloat16

B_PER_CORE = 4
N_SUP = 25
C = 512
HW = 196
NIJ = N_SUP * HW  # 4900
DK = 128
NCH = (NIJ + 127) // 128  # 39 chunks of <=128 along nij
FT = 490                   # nij tile width for projections (fits one PSUM bank)
NT = NIJ // FT             # 10
CCH = C // 128             # 4 c-chunks


def _spans(start, end):
    """Split nij range [start,end) into DMA pieces aligned to n boundaries.
    Returns (n0, nn, ij0, L, dstoff) tuples; full-n middle merged into one."""
    res = []
    cur = start
    if cur % HW != 0:
        n = cur // HW
        ij0 = cur % HW
        L = min(HW - ij0, end - cur)
        res.append((n, 1, ij0, L, cur - start))
        cur += L
    nfull = (end - cur) // HW
    if nfull > 0:
        res.append((cur // HW, nfull, 0, HW, cur - start))
        cur += nfull * HW
    if cur < end:
        res.append((cur // HW, 1, 0, end - cur, cur - start))
    return res


def build_bass():
    nc = bacc.Bacc(
        "TRN2", target_bir_lowering=False, debug=False, enable_asserts=False
    )
    q_d = nc.dram_tensor("q", [B_PER_CORE, C, HW], F32, kind="ExternalInput").ap()
    s_d = nc.dram_tensor(
        "s", [B_PER_CORE, N_SUP, C, HW], F32, kind="ExternalInput"
    ).ap()
    wqk_d = nc.dram_tensor("wqk", [DK, C], F32, kind="ExternalInput").ap()
    wv_d = nc.dram_tensor("wv", [DK, C], F32, kind="ExternalInput").ap()
    out_d = nc.dram_tensor("out", [1, 1], F32, kind="ExternalOutput").ap()

    with tile.TileContext(nc) as tc:
        with (
            tc.tile_pool(name="const", bufs=1) as const,
            tc.tile_pool(name="spool", bufs=16) as spool,
            tc.tile_pool(name="kvbf", bufs=2) as kvbf,
            tc.tile_pool(name="vt1p", bufs=2 * NCH + 4) as vt1p,
            tc.tile_pool(name="etp", bufs=NCH + 5) as etp,
            tc.tile_pool(name="small", bufs=4) as small,
            tc.tile_pool(name="ps_proj", bufs=3, space="PSUM") as ps_proj,
            tc.tile_pool(name="ps_sim", bufs=2, space="PSUM") as ps_sim,
            tc.tile_pool(name="ps_vt", bufs=2, space="PSUM") as ps_vt,
            tc.tile_pool(name="ps_ctx", bufs=1, space="PSUM") as ps_ctx,
        ):
            # ---- constants / weights ----
            id_f32 = const.tile([128, 128], F32, tag="id_f32")
            make_identity(nc, id_f32)
            id_bf = const.tile([128, 128], BF16, tag="id_bf")
            make_identity(nc, id_bf)

            wqk_sb = const.tile([128, C], F32, tag="wqk_sb")
            nc.sync.dma_start(out=wqk_sb, in_=wqk_d)
            wv_sb = const.tile([128, C], F32, tag="wv_sb")
            nc.sync.dma_start(out=wv_sb, in_=wv_d)

            wqkT = []
            wvT = []
            for cc in range(CCH):
                for (src, dstl, nm) in ((wqk_sb, wqkT, "qk"), (wv_sb, wvT, "v")):
                    pt = ps_vt.tile([128, 128], F32, tag="ps_vt")
                    nc.tensor.transpose(pt, src[:, cc * 128 : (cc + 1) * 128], id_f32)
                    wt = const.tile([128, 128], F32R, tag=f"w{nm}T{cc}")
                    nc.vector.tensor_copy(wt, pt)
                    dstl.append(wt)

            # ---- query load + projections (all 4 batches at once) ----
            qsb = []
            for cc in range(CCH):
                qt = const.tile([128, B_PER_CORE * HW], F32R, tag=f"qsb{cc}")
                src = q_d[:, cc * 128 : (cc + 1) * 128, :].rearrange(
                    "b c ij -> c b ij"
                ).bitcast(F32R)
                nc.sync.dma_start(
                    out=qt.rearrange("p (b ij) -> p b ij", b=B_PER_CORE), in_=src
                )
                qsb.append(qt)

            qq_bf = const.tile([128, B_PER_CORE * HW], BF16, tag="qq_bf")
            qv_sb = const.tile([128, B_PER_CORE * HW], F32, tag="qv_sb")
            for wT, dst in ((wqkT, qq_bf), (wvT, qv_sb)):
                for half in range(2):
                    hw0 = half * 392
                    pq = ps_proj.tile([128, FT], F32, tag="ps_proj")
                    for cc in range(CCH):
                        nc.tensor.matmul(
                            pq[:, :392],
                            lhsT=wT[cc],
                            rhs=qsb[cc][:, hw0 : hw0 + 392],
                            start=(cc == 0),
                            stop=(cc == CCH - 1),
                        )
                    nc.vector.tensor_copy(dst[:, hw0 : hw0 + 392], pq[:, :392])

            # qv^T per (b, hw-chunk): [hw<=128, 128] fp32 — matches ctx layout
            qvT = {}
            for b in range(B_PER_CORE):
                for h in range(2):
                    hww = 128 if h == 0 else HW - 128
                    pt = ps_vt.tile([128, 128], F32, tag="ps_vt")
                    nc.tensor.transpose(
                        pt[:hww, :],
                        qv_sb[:, b * HW + h * 128 : b * HW + h * 128 + hww],
                        id_f32,
                    )
                    qt = const.tile([128, 128], F32, tag=f"qvT{b}_{h}")
                    nc.vector.tensor_copy(qt[:hww, :], pt[:hww, :])
                    qvT[(b, h)] = qt

            partials = const.tile([128, 2 * B_PER_CORE], F32, tag="partials")
            nc.vector.memset(partials, 0.0)

            # ---- per-batch main pipeline ----
            import os as _os
            KPHASES = int(_os.environ.get("KPHASES", "4"))
            for b in range(B_PER_CORE):
                k_bf = kvbf.tile([128, NIJ], BF16, tag="k_bf")
                v_bf = kvbf.tile([128, NIJ], BF16, tag="v_bf")

                # projections: stream S in FT-wide nij tiles
                for t in range(NT):
                    st = []
                    for cc in range(CCH):
                        s_t = spool.tile([128, FT], F32R, tag="s_t")
                        for (n0, nn, ij0, L, off) in _spans(t * FT, (t + 1) * FT):
                            src = s_d[
                                b, n0 : n0 + nn, cc * 128 : (cc + 1) * 128,
                                ij0 : ij0 + L,
                            ].rearrange("n c ij -> c n ij").bitcast(F32R)
                            nc.sync.dma_start(
                                out=s_t[:, off : off + nn * L].rearrange(
                                    "p (n ij) -> p n ij", n=nn
                                ),
                                in_=src,
                            )
                        st.append(s_t)
                    pk = ps_proj.tile([128, FT], F32, tag="ps_proj")
                    for cc in range(CCH):
                        nc.tensor.matmul(
                            pk,
                            lhsT=wqkT[cc],
                            rhs=st[cc],
                            start=(cc == 0),
                            stop=(cc == CCH - 1),
                        )
                    nc.vector.tensor_copy(k_bf[:, t * FT : (t + 1) * FT], pk)
                    pv = ps_proj.tile([128, FT], F32, tag="ps_proj")
                    for cc in range(CCH):
                        nc.tensor.matmul(
                            pv,
                            lhsT=wvT[cc],
                            rhs=st[cc],
                            start=(cc == 0),
                            stop=(cc == CCH - 1),
                        )
                    nc.scalar.copy(v_bf[:, t * FT : (t + 1) * FT], pv)

                # V^T chunks (+ ones column) via PE transpose
                vt1 = []
                if KPHASES < 2:
                    continue
                for j in range(NCH):
                    cw = min(128, NIJ - j * 128)
                    vt = vt1p.tile([128, 132], BF16, tag="vt1")
                    if cw < 128:
                        nc.vector.memset(vt, 0.0)
                    pt = ps_vt.tile([128, 128], BF16, tag="ps_vt")
                    nc.tensor.transpose(
                        pt[:cw, :], v_bf[:, j * 128 : j * 128 + cw], id_bf
                    )
                    nc.vector.tensor_copy(vt[:cw, 0:128], pt[:cw, :])
                    nc.vector.memset(vt[:, 128:132], 1.0)
                    vt1.append(vt)

                # simT = K^T @ qq (bf16), exp -> E^T chunks
                et = []
                if KPHASES < 3:
                    continue
                for j in range(NCH):
                    cw = min(128, NIJ - j * 128)
                    ps = ps_sim.tile([128, HW], F32, tag="ps_sim")
                    nc.tensor.matmul(
                        ps[:cw, :],
                        lhsT=k_bf[:, j * 128 : j * 128 + cw],
                        rhs=qq_bf[:, b * HW : (b + 1) * HW],
                        start=True,
                        stop=True,
                    )
                    e = etp.tile([128, HW], BF16, tag="et")
                    if cw < 128:
                        nc.vector.memset(e, 0.0)
                    nc.scalar.activation(
                        out=e[:cw, :],
                        in_=ps[:cw, :],
                        func=mybir.ActivationFunctionType.Exp,
                    )
                    et.append(e)

                # PV: ctx_raw[hw, 129] accumulated over 39 nij chunks
                if KPHASES < 4:
                    continue
                for h in range(2):
                    hww = 128 if h == 0 else HW - 128
                    pc = ps_ctx.tile([128, 132], F32, tag="ps_ctx")
                    for j in range(NCH):
                        nc.tensor.matmul(
                            pc[:hww, 0:132],
                            lhsT=et[j][:, h * 128 : h * 128 + hww],
                            rhs=vt1[j][:, 0:132],
                            start=(j == 0),
                            stop=(j == NCH - 1),
                        )
                    r = small.tile([128, 1], F32, tag="recip")
                    nc.vector.reciprocal(r[:hww], pc[:hww, 128:129])
                    ctx = small.tile([128, 128], F32, tag="ctx")
                    nc.vector.tensor_scalar_mul(
                        ctx[:hww, :], pc[:hww, 0:128], r[:hww]
                    )
                    d = small.tile([128, 128], F32, tag="diff")
                    nc.vector.tensor_sub(
                        d[:hww, :], qvT[(b, h)][:hww, :], ctx[:hww, :]
                    )
                    d2 = small.tile([128, 128], F32, tag="d2")
                    nc.vector.tensor_mul(d2[:hww, :], d[:hww, :], d[:hww, :])
                    nc.vector.reduce_sum(
                        partials[:hww, 2 * b + h : 2 * b + h + 1],
                        d2[:hww, :],
                        axis=mybir.AxisListType.X,
                    )

            # ---- final reduction to scalar ----
            tot = small.tile([128, 1], F32, tag="tot")
            nc.vector.reduce_sum(tot, partials, axis=mybir.AxisListType.X)
            ones = small.tile([128, 1], F32, tag="ones")
            nc.vector.memset(ones, 1.0)
            pf = ps_vt.tile([128, 128], F32, tag="ps_vt")
            nc.tensor.matmul(pf[0:1, 0:1], lhsT=tot, rhs=ones, start=True, stop=True)
            ob = small.tile([1, 1], F32, tag="ob")
            nc.vector.tensor_copy(ob, pf[0:1, 0:1])
            nc.sync.dma_start(out=out_d, in_=ob)

    nc.compile()
    return nc


_NC = None


def kernel(query_repr, supports_repr, W_qk, W_v):
    global _NC
    q = np.ascontiguousarray(np.asarray(query_repr, dtype=np.float32)).reshape(
        32, C, HW
    )
    s = np.ascontiguousarray(np.asarray(supports_repr, dtype=np.float32)).reshape(
        32, N_SUP, C, HW
    )
    wqk = np.ascontiguousarray(np.asarray(W_qk, dtype=np.float32))
    wv = np.ascontiguousarray(np.asarray(W_v, dtype=np.float32))

    if _NC is None:
        _NC = build_bass()

    in_maps = []
    for core in range(8):
        b0 = core * B_PER_CORE
        in_maps.append(
            {
                "q": np.ascontiguousarray(q[b0 : b0 + B_PER_CORE]),
                "s": np.ascontiguousarray(s[b0 : b0 + B_PER_CORE]),
                "wqk": wqk,
                "wv": wv,
            }
        )
    res = run_bass_kernel_spmd(
        _NC, in_maps, core_ids=list(range(8)),
        trace=bool(int(os.environ.get("KTRACE", "0"))),
    )
    total = sum(float(r["out"][0, 0]) for r in res.results) / float(HW)
    kernel._last_results = res
    return np.asarray(total, dtype=np.float32)



# revision 2
# speedup vs baseline: 3.7821x; 3.7821x over previous
"""CrossTransformer kernel v4 for Trainium2 — fp8 sim-direct, direct-V^T.

Per batch b (B=32 -> 4/core, N=25, C=512, H=W=14, DK=DV=128):
  qq = Wqk @ Q   (bf16)        qv = Wv @ Q  (bf16 -> f32)
  qk = Wqk^T @ qq  -> fp8 e4m3 DoubleRow layout [g][p][t][hw], c = g*256+t*128+p
  sim[nij,hw] = S^T @ qk       (fp8 DoubleRow; S is never projected to K)
  E = exp(sim) bf16            (ACT, 392-wide PSUM chunk pairs)
  V^T[nij,dv] = S^T @ (16*Wv)^T  (fp8 DoubleRow, direct transposed layout)
  ctx_raw[hw,132] = sum_j E_j^T @ [V^T_j | 16]   (ones=16 cancels the Wv scale)
  partial += sum((qv^T - num*recip(den))^2)

Schedule: iteration b emits sim(b) interleaved with V^T(b+1), qk(b+1) and
PV(b-1) (both halves, 2 matmuls each per slot) so the PE never head-of-line
blocks on exp or PSUM drains; the ACT exp stream is the pacing engine.
Batch 3's PV runs in-iteration lagging its own exp stream by 2 slots.
"""

import os
import sys

sys.path.insert(0, "/opt/trn_rl_repo")

import numpy as np
import ml_dtypes

import concourse.bass as bass
import concourse.bacc as bacc
import concourse.mybir as mybir
import concourse.tile as tile
from concourse.bass_utils import run_bass_kernel_spmd
from concourse.masks import make_identity

F32 = mybir.dt.float32
BF16 = mybir.dt.bfloat16
FP8 = mybir.dt.float8e4

B_PER_CORE = 4
N_SUP = 25
C = 512
HW = 196
NIJ = N_SUP * HW          # 4900
DK = 128
NCH = 39                  # nij chunks of <=128 (last = 36 rows)
NIJP = 4912               # s8 SBUF row pitch: 16B-aligned for DoubleRow
HWP = 208                 # qk8 row pitch: 16B-aligned for DoubleRow
NPAIR = 20                # sim chunk pairs (last = chunk 38 alone)
VSCALE = 16.0             # host scales Wv by 16; ones column = 16 cancels it

DR = mybir.MatmulPerfMode.DoubleRow
EXP = mybir.ActivationFunctionType.Exp
RCP = mybir.ActivationFunctionType.Reciprocal
MULT = mybir.AluOpType.mult
SUBTRACT = mybir.AluOpType.subtract
ADD = mybir.AluOpType.add


def build_bass():
    nc = bacc.Bacc(
        "TRN2", target_bir_lowering=False, debug=False, enable_asserts=False
    )
    s8_d = nc.dram_tensor(
        "s8", [B_PER_CORE, 2, 128, 2, NIJ], FP8, kind="ExternalInput"
    ).ap()
    # packed: [p, cc0..3]=WqkT chunks, [p, 4..7]=WvT chunks
    wT_d = nc.dram_tensor("wT", [128, 8, DK], BF16, kind="ExternalInput").ap()
    q_d = nc.dram_tensor(
        "qbf", [128, 4, B_PER_CORE * HW], BF16, kind="ExternalInput"
    ).ap()
    wqk_d = nc.dram_tensor("wqk", [DK, C], BF16, kind="ExternalInput").ap()
    wv8_d = nc.dram_tensor("wv8", [128, 2, 2, DK], FP8, kind="ExternalInput").ap()
    out_d = nc.dram_tensor("out", [1, 1], F32, kind="ExternalOutput").ap()

    with tile.TileContext(nc) as tc:
        with (
            tc.tile_pool(name="const", bufs=1) as const,
            tc.tile_pool(name="s8p", bufs=8) as s8p,
            tc.tile_pool(name="etp", bufs=2) as etp,
            tc.tile_pool(name="vtp", bufs=3) as vtp,
            tc.tile_pool(name="qk8p", bufs=2) as qk8p,
            tc.tile_pool(name="small", bufs=8) as small,
            tc.tile_pool(name="ps_sim", bufs=4, space="PSUM") as ps_sim,
            tc.tile_pool(name="ps_vt", bufs=2, space="PSUM") as ps_vt,
            tc.tile_pool(name="ps_ctx", bufs=2, space="PSUM") as ps_ctx,
        ):
            # ---- input DMAs, ordered for fastest time-to-first-exp ----
            wT_sb = const.tile([128, 8, DK], BF16, tag="wT_sb")
            nc.sync.dma_start(out=wT_sb, in_=wT_d)
            q_sb = const.tile([128, 4, B_PER_CORE * HW], BF16, tag="q_sb")
            nc.sync.dma_start(out=q_sb, in_=q_d)
            wqk_sb = const.tile([128, C], BF16, tag="wqk_sb")
            nc.sync.dma_start(out=wqk_sb, in_=wqk_d)

            s8 = {}

            def s8_alloc(b):
                for g in range(2):
                    s8t = s8p.tile([128, 2, NIJP], FP8, tag="s8")
                    s8[(b, g)] = s8t

            def s8_piece(b, o, ln):
                for g in range(2):
                    nc.sync.dma_start(
                        out=s8[(b, g)][:, :, o : o + ln],
                        in_=s8_d[b, g][:, :, o : o + ln],
                    )

            def s8_dma(b, pieces):
                s8_alloc(b)
                w = NIJ // pieces
                for i in range(pieces):
                    o = i * w
                    s8_piece(b, o, w if i < pieces - 1 else NIJ - o)

            # prologue pieces ordered so sim(0)/vt(0)/vt(1) prerequisites
            # land just in time (HWDGE serializes at 625ns per DMA)
            s8_alloc(0)
            s8_alloc(1)
            s8_piece(0, 0, 1225)
            wv8_sb = const.tile([128, 2, 2, DK], FP8, tag="wv8_sb")
            nc.sync.dma_start(out=wv8_sb, in_=wv8_d)
            s8_piece(1, 0, 2450)
            s8_piece(0, 1225, 1225)
            s8_piece(0, 2450, 2450)
            s8_piece(1, 2450, 2450)

            # PE p-state warmup: wide matmuls on a zero tile keep the PE
            # continuously busy through the DMA wait so the real prologue
            # runs at full clock (ap 512 > write latency -> no WAW stall)
            warm_src = const.tile([128, 512], BF16, tag="warm_src")
            nc.gpsimd.memset(warm_src, 0.0)
            for i in range(14):
                pw = ps_vt.tile([128, 512], F32, tag="ps_vt")
                nc.tensor.matmul(
                    pw,
                    lhsT=warm_src[:, 0:128],
                    rhs=warm_src,
                    start=True,
                    stop=True,
                )


            # ---- qq projection (all 4 batches at once) ----
            qq_bf = const.tile([128, B_PER_CORE * HW], BF16, tag="qq_bf")

            def q_proj(wo, dst, eng):
                for half in range(2):
                    hw0 = half * 392
                    pq = ps_sim.tile([128, 392], F32, tag="ps_sim")
                    for cc in range(4):
                        nc.tensor.matmul(
                            pq,
                            lhsT=wT_sb[:, wo + cc],
                            rhs=q_sb[:, cc, hw0 : hw0 + 392],
                            start=(cc == 0),
                            stop=(cc == 3),
                        )
                    eng.tensor_copy(dst[:, hw0 : hw0 + 392], pq)

            qvT = {}

            def qvT_prep(b, h):
                """qv^T[hw, dk] computed directly: lhsT = Q chunk, rhs = Wv^T."""
                hww = 128 if h == 0 else HW - 128
                o = b * HW + h * 128
                pt = ps_sim.tile([128, 392], F32, tag="ps_sim")
                for cc in range(4):
                    nc.tensor.matmul(
                        pt[:hww, 0:128],
                        lhsT=q_sb[:, cc, o : o + hww],
                        rhs=wT_sb[:, 4 + cc],
                        start=(cc == 0),
                        stop=(cc == 3),
                    )
                qt = const.tile([128, 128], F32, tag=f"qvT{b}_{h}")
                nc.vector.tensor_copy(qt[:hww, :], pt[:hww, 0:128])
                qvT[(b, h)] = qt


            # ---- per-batch stage generators (interleavable) ----
            et = {}
            vt1 = {}
            qk8 = {}

            def qk_prep(b):
                """qk = Wqk^T @ qq -> fp8 DoubleRow layout [128, 2(g), 2(t), 196]."""
                k8 = qk8p.tile([128, 2, 2, HWP], FP8, tag="qk8")
                for g in range(2):
                    pk = ps_sim.tile([128, 392], F32, tag="ps_sim")
                    for t in range(2):
                        cc = 2 * g + t
                        nc.tensor.matmul(
                            pk[:, t * HW : (t + 1) * HW],
                            lhsT=wqk_sb[:, cc * 128 : (cc + 1) * 128],
                            rhs=qq_bf[:, b * HW : (b + 1) * HW],
                            start=True,
                            stop=True,
                        )
                    nc.vector.tensor_copy(
                        k8[:, g, :, 0:HW],
                        pk.rearrange("p (t hw) -> p t hw", t=2),
                    )
                qk8[b] = k8

            def sim_pair_gen(b):
                """Yield once per sim chunk pair: 4 matmuls + 1 exp on ACT."""
                e = etp.tile([128, NCH * HW], BF16, tag="et")
                nc.gpsimd.memset(e[:, 38 * HW :], 0.0)
                et[b] = e
                for jp in range(NPAIR):
                    chunks = (2 * jp, 2 * jp + 1) if jp < NPAIR - 1 else (38,)
                    ps = ps_sim.tile([128, 392], F32, tag="ps_sim")
                    for ci, j in enumerate(chunks):
                        cw = min(128, NIJ - j * 128)
                        for g in range(2):
                            nc.tensor.matmul(
                                ps[:cw, ci * HW : (ci + 1) * HW],
                                lhsT=s8[(b, g)][:, :, j * 128 : j * 128 + cw],
                                rhs=qk8[b][:, g, :, 0:HW],
                                start=(g == 0),
                                stop=(g == 1),
                                perf_mode=DR,
                            )
                    if jp < NPAIR - 1:
                        nc.scalar.activation(
                            out=e[:, 2 * jp * HW : (2 * jp + 2) * HW],
                            in_=ps,
                            func=EXP,
                        )
                    else:
                        nc.scalar.activation(
                            out=e[:36, 38 * HW :], in_=ps[:36, 0:HW], func=EXP
                        )
                    yield

            def vt_alloc(b):
                vt = vtp.tile([128, NCH * 132], BF16, tag="vt1")
                nc.gpsimd.memset(
                    vt.rearrange("p (j c) -> p j c", j=NCH)[:, :, 128:132], VSCALE
                )
                vt1[b] = vt

            def vt_gen(b, j_lo, j_hi, copy_eng0):
                """Yield per V^T chunk: 2 DoubleRow matmuls into a [128,512]
                PSUM quad tile; one wide copy per quad."""
                vtr = vt1[b].rearrange("p (j c) -> p j c", j=NCH)
                ncopy = copy_eng0
                for j0 in range(j_lo, j_hi, 4):
                    jn = min(4, j_hi - j0)
                    pq = ps_vt.tile([128, 512], F32, tag="ps_vt")
                    for ji in range(jn):
                        j = j0 + ji
                        cw = min(128, NIJ - j * 128)
                        for g in range(2):
                            nc.tensor.matmul(
                                pq[:cw, ji * 128 : (ji + 1) * 128],
                                lhsT=s8[(b, g)][:, :, j * 128 : j * 128 + cw],
                                rhs=wv8_sb[:, g],
                                start=(g == 0),
                                stop=(g == 1),
                                perf_mode=DR,
                            )
                        yield
                    nc.vector.tensor_copy(
                        vtr[:, j0 : j0 + jn, 0:128],
                        pq[:, 0 : jn * 128].rearrange("p (j c) -> p j c", j=jn),
                    )
                    ncopy += 1

            def pv_half_gen(b, h):
                """Yield per PV matmul (one et chunk each)."""
                vtr = vt1[b].rearrange("p (j c) -> p j c", j=NCH)
                hww = 128 if h == 0 else HW - 128
                pc = ps_ctx.tile([128, 132], F32, tag="ps_ctx")
                for j in range(NCH):
                    nc.tensor.matmul(
                        pc[:hww, :],
                        lhsT=et[b][:, j * HW + h * 128 : j * HW + h * 128 + hww],
                        rhs=vtr[:, j],
                        start=(j == 0),
                        stop=(j == NCH - 1),
                    )
                    yield
                # ctx epilogue avoids DVE entirely: late-run DVE queues behind
                # the tile framework's semaphore range-clear barriers.
                r = small.tile([128, 1], F32, tag="recip")
                nc.vector.reciprocal(r[:hww], pc[:hww, 128:129])
                d = small.tile([128, 128], F32, tag="diff")
                nc.vector.scalar_tensor_tensor(
                    d[:hww, :],
                    pc[:hww, 0:128],
                    r[:hww],
                    qvT[(b, h)][:hww, :],
                    op0=MULT,
                    op1=SUBTRACT,
                )
                d2 = small.tile([128, 128], F32, tag="d2")
                nc.vector.scalar_tensor_tensor(
                    d2[:hww, :],
                    d[:hww, :],
                    1.0,
                    d[:hww, :],
                    op0=MULT,
                    op1=MULT,
                    accum_out=partials[:hww, 2 * b + h : 2 * b + h + 1],
                )

            def drain(gen, n=None):
                if gen is None:
                    return None
                try:
                    if n is None:
                        while True:
                            next(gen)
                    else:
                        for _ in range(n):
                            next(gen)
                except StopIteration:
                    return None
                return gen

            # ---- schedule ----
            # PV(b) runs in-iteration, lagging its own exp stream by 2 slots;
            # its last 3 matmuls + ctx epilogue carry into iteration b+1.
            partials = const.tile([128, 2 * B_PER_CORE], F32, tag="partials")
            nc.vector.memset(partials, 0.0)
            q_proj(0, qq_bf, nc.vector)
            qk_prep(0)
            for i, (bq, hq) in enumerate(
                (b, h) for b in range(B_PER_CORE) for h in range(2)
            ):
                qvT_prep(bq, hq)
            vt_alloc(0)
            drain(vt_gen(0, 0, 19, 0))   # chunks covered by first two s8 quarters
            vt0_rest = vt_gen(0, 19, NCH, 1)
            carry = []

            for b in range(B_PER_CORE):
                simg = sim_pair_gen(b)
                if b + 1 < B_PER_CORE:
                    vt_alloc(b + 1)
                    vtg = vt_gen(b + 1, 0, NCH, 0)
                else:
                    vtg = None
                pvg = [pv_half_gen(b, h) for h in range(2)]
                for jp in range(NPAIR):
                    simg = drain(simg, 1)
                    if b == 0:
                        vt0_rest = drain(vt0_rest, 2)
                    if jp < 2 and carry:
                        carry = [drain(g, 2) for g in carry if g is not None]
                    if b > 0 or jp >= 2:
                        vtg = drain(vtg, 3 if b == 0 else 2)
                    if jp >= 1:
                        pvg = [drain(g, 2) for g in pvg]
                    if jp == 0 and b + 2 < B_PER_CORE:
                        s8_dma(b + 2, 2)
                    if jp == 4 and b + 1 < B_PER_CORE:
                        qk_prep(b + 1)
                drain(simg)
                drain(vtg)
                if b == 0:
                    drain(vt0_rest)
                for g in carry:
                    drain(g)
                carry = [g for g in pvg if g is not None]
            for g in carry:
                drain(g)

            # ---- final reduction to scalar ----
            ob = small.tile([1, 1], F32, tag="ob")
            nc.gpsimd.tensor_reduce(
                ob, partials, axis=mybir.AxisListType.XYZWC,
                op=mybir.AluOpType.add,
            )
            nc.sync.dma_start(out=out_d, in_=ob)

    nc.compile()
    return nc


_NC = None


def _prep_core(q, s, b0):
    """Host-side per-core input layouts (cast + transpose only)."""
    F8NP = ml_dtypes.float8_e4m3
    BFNP = ml_dtypes.bfloat16
    sb = s[b0 : b0 + B_PER_CORE]                      # [4, 25, 512, 196]
    s_c = sb.transpose(0, 2, 1, 3).reshape(B_PER_CORE, 2, 2, 128, NIJ)
    s8 = np.ascontiguousarray(s_c.transpose(0, 1, 3, 2, 4)).astype(F8NP)
    qb = q[b0 : b0 + B_PER_CORE]                      # [4, 512, 196]
    qbf = np.ascontiguousarray(
        qb.reshape(B_PER_CORE, 4, 128, HW).transpose(2, 1, 0, 3).reshape(
            128, 4, B_PER_CORE * HW
        )
    ).astype(BFNP)
    return {"s8": s8, "qbf": qbf}


def kernel(query_repr, supports_repr, W_qk, W_v):
    global _NC
    F8NP = ml_dtypes.float8_e4m3
    BFNP = ml_dtypes.bfloat16

    q = np.asarray(query_repr, dtype=np.float32).reshape(32, C, HW)
    s = np.asarray(supports_repr, dtype=np.float32).reshape(32, N_SUP, C, HW)
    wqk = np.asarray(W_qk, dtype=np.float32)
    wv = np.asarray(W_v, dtype=np.float32)

    wqk_bf = wqk.astype(BFNP)
    # wT[p, 0:4, dk] = Wqk^T chunks, wT[p, 4:8, dk] = Wv^T chunks
    wT = np.concatenate(
        [wqk.T.reshape(4, 128, DK), wv.T.reshape(4, 128, DK)], axis=0
    ).transpose(1, 0, 2)
    wT = np.ascontiguousarray(wT).astype(BFNP)
    wv8 = np.ascontiguousarray(
        (VSCALE * wv).T.reshape(2, 2, 128, DK).transpose(2, 0, 1, 3)
    ).astype(F8NP)

    if _NC is None:
        _NC = build_bass()

    in_maps = []
    for core in range(8):
        m = _prep_core(q, s, core * B_PER_CORE)
        m.update({"wqk": wqk_bf, "wT": wT, "wv8": wv8})
        in_maps.append(m)
    res = run_bass_kernel_spmd(
        _NC, in_maps, core_ids=list(range(8)),
        trace=bool(int(os.environ.get("KTRACE", "0"))),
    )
    total = sum(float(r["out"][0, 0]) for r in res.results) / float(HW)
    kernel._last_results = res
    return np.asarray(total, dtype=np.float32)


# revision 3
# speedup vs baseline: 3.8219x; 1.0105x over previous
"""CrossTransformer kernel v4 for Trainium2 — fp8 sim-direct, direct-V^T.

Per batch b (B=32 -> 4/core, N=25, C=512, H=W=14, DK=DV=128):
  qq = Wqk @ Q   (bf16)        qv = Wv @ Q  (bf16 -> f32)
  qk = Wqk^T @ qq  -> fp8 e4m3 DoubleRow layout [g][p][t][hw], c = g*256+t*128+p
  sim[nij,hw] = S^T @ qk       (fp8 DoubleRow; S is never projected to K)
  E = exp(sim) bf16            (ACT, 392-wide PSUM chunk pairs)
  V^T[nij,dv] = S^T @ (16*Wv)^T  (fp8 DoubleRow, direct transposed layout)
  ctx_raw[hw,132] = sum_j E_j^T @ [V^T_j | 16]   (ones=16 cancels the Wv scale)
  partial += sum((qv^T - num*recip(den))^2)

Schedule: iteration b emits sim(b) interleaved with V^T(b+1), qk(b+1) and
PV(b) (both halves, 2 matmuls per slot, lagging the exp stream by 1 slot;
tails carry into iteration b+1) so the PE never head-of-line blocks on exp
or PSUM drains; the ACT exp stream (~10.4us/batch) is the pacing engine.
GPSIMD touches only SBUF (PSUM access is rejected by the BIR verifier) and
DoubleRow operand k-tile strides are padded to 16B alignment (4912/208).
"""

import os
import sys

sys.path.insert(0, "/opt/trn_rl_repo")

import numpy as np
import ml_dtypes

import concourse.bass as bass
import concourse.bacc as bacc
import concourse.mybir as mybir
import concourse.tile as tile
from concourse.bass_utils import run_bass_kernel_spmd
from concourse.masks import make_identity

F32 = mybir.dt.float32
BF16 = mybir.dt.bfloat16
FP8 = mybir.dt.float8e4

B_PER_CORE = 4
N_SUP = 25
C = 512
HW = 196
NIJ = N_SUP * HW          # 4900
DK = 128
NCH = 39                  # nij chunks of <=128 (last = 36 rows)
NIJP = 4912               # s8 SBUF row pitch: 16B-aligned for DoubleRow
HWP = 208                 # qk8 row pitch: 16B-aligned for DoubleRow
NPAIR = 20                # sim chunk pairs (last = chunk 38 alone)
VSCALE = 16.0             # host scales Wv by 16; ones column = 16 cancels it

DR = mybir.MatmulPerfMode.DoubleRow
EXP = mybir.ActivationFunctionType.Exp
RCP = mybir.ActivationFunctionType.Reciprocal
MULT = mybir.AluOpType.mult
SUBTRACT = mybir.AluOpType.subtract
ADD = mybir.AluOpType.add


def build_bass():
    nc = bacc.Bacc(
        "TRN2", target_bir_lowering=False, debug=False, enable_asserts=False
    )
    s8_d = nc.dram_tensor(
        "s8", [B_PER_CORE, 2, 128, 2, NIJ], FP8, kind="ExternalInput"
    ).ap()
    # packed: [p, cc0..3]=WqkT chunks, [p, 4..7]=WvT chunks
    wT_d = nc.dram_tensor("wT", [128, 8, DK], BF16, kind="ExternalInput").ap()
    q_d = nc.dram_tensor(
        "qbf", [128, 4, B_PER_CORE * HW], BF16, kind="ExternalInput"
    ).ap()
    wqk_d = nc.dram_tensor("wqk", [DK, C], BF16, kind="ExternalInput").ap()
    wv8_d = nc.dram_tensor("wv8", [128, 2, 2, DK], FP8, kind="ExternalInput").ap()
    out_d = nc.dram_tensor("out", [1, 1], F32, kind="ExternalOutput").ap()

    with tile.TileContext(nc) as tc:
        with (
            tc.tile_pool(name="const", bufs=1) as const,
            tc.tile_pool(name="s8p", bufs=8) as s8p,
            tc.tile_pool(name="etp", bufs=2) as etp,
            tc.tile_pool(name="vtp", bufs=3) as vtp,
            tc.tile_pool(name="qk8p", bufs=4) as qk8p,
            tc.tile_pool(name="small", bufs=8) as small,
            tc.tile_pool(name="ps_sim", bufs=4, space="PSUM") as ps_sim,
            tc.tile_pool(name="ps_vt", bufs=2, space="PSUM") as ps_vt,
            tc.tile_pool(name="ps_ctx", bufs=2, space="PSUM") as ps_ctx,
        ):
            # ---- input DMAs, ordered for fastest time-to-first-exp ----
            wT_sb = const.tile([128, 8, DK], BF16, tag="wT_sb")
            nc.sync.dma_start(out=wT_sb, in_=wT_d)
            q_sb = const.tile([128, 4, B_PER_CORE * HW], BF16, tag="q_sb")
            nc.sync.dma_start(out=q_sb[:, 0:2], in_=q_d[:, 0:2])
            nc.sync.dma_start(out=q_sb[:, 2:4], in_=q_d[:, 2:4])
            wqk_sb = const.tile([128, C], BF16, tag="wqk_sb")
            nc.sync.dma_start(out=wqk_sb, in_=wqk_d)

            s8 = {}

            def s8_alloc(b):
                for g in range(2):
                    s8t = s8p.tile([128, 2, NIJP], FP8, tag="s8")
                    s8[(b, g)] = s8t

            def s8_piece(b, o, ln):
                for g in range(2):
                    nc.sync.dma_start(
                        out=s8[(b, g)][:, :, o : o + ln],
                        in_=s8_d[b, g][:, :, o : o + ln],
                    )

            def s8_dma(b, pieces):
                s8_alloc(b)
                w = NIJ // pieces
                for i in range(pieces):
                    o = i * w
                    s8_piece(b, o, w if i < pieces - 1 else NIJ - o)

            # prologue pieces ordered so sim(0)/vt(0)/vt(1) prerequisites
            # land just in time (HWDGE serializes at 625ns per DMA)
            s8_alloc(0)
            s8_alloc(1)
            s8_piece(0, 0, 1225)
            wv8_sb = const.tile([128, 2, 2, DK], FP8, tag="wv8_sb")
            nc.sync.dma_start(out=wv8_sb, in_=wv8_d)
            s8_piece(1, 0, 2450)
            s8_piece(0, 1225, 1225)
            s8_piece(0, 2450, 2450)
            s8_piece(1, 2450, 2450)

            # PE p-state warmup: wide matmuls on a zero tile keep the PE
            # continuously busy through the DMA wait so the real prologue
            # runs at full clock (ap 512 > write latency -> no WAW stall)
            warm_src = const.tile([128, 512], BF16, tag="warm_src")
            nc.gpsimd.memset(warm_src, 0.0)
            for i in range(12):
                pw = ps_vt.tile([128, 512], F32, tag="ps_vt")
                nc.tensor.matmul(
                    pw,
                    lhsT=warm_src[:, 0:128],
                    rhs=warm_src,
                    start=True,
                    stop=True,
                )


            # ---- qq projection (all 4 batches at once) ----
            qq_bf = const.tile([128, B_PER_CORE * HW], BF16, tag="qq_bf")

            def q_proj(wo, dst, eng):
                for half in range(2):
                    hw0 = half * 392
                    pq = ps_sim.tile([128, 392], F32, tag="ps_sim")
                    for cc in range(4):
                        nc.tensor.matmul(
                            pq,
                            lhsT=wT_sb[:, wo + cc],
                            rhs=q_sb[:, cc, hw0 : hw0 + 392],
                            start=(cc == 0),
                            stop=(cc == 3),
                        )
                    eng.tensor_copy(dst[:, hw0 : hw0 + 392], pq)

            qvT = {}

            def qvT_prep(b, h):
                """qv^T[hw, dk] computed directly: lhsT = Q chunk, rhs = Wv^T."""
                hww = 128 if h == 0 else HW - 128
                o = b * HW + h * 128
                pt = ps_sim.tile([128, 392], F32, tag="ps_sim")
                for cc in range(4):
                    nc.tensor.matmul(
                        pt[:hww, 0:128],
                        lhsT=q_sb[:, cc, o : o + hww],
                        rhs=wT_sb[:, 4 + cc],
                        start=(cc == 0),
                        stop=(cc == 3),
                    )
                qt = const.tile([128, 128], F32, tag=f"qvT{b}_{h}")
                nc.vector.tensor_copy(qt[:hww, :], pt[:hww, 0:128])
                qvT[(b, h)] = qt


            # ---- per-batch stage generators (interleavable) ----
            et = {}
            vt1 = {}
            qk8 = {}

            def qk_prep(b):
                """qk = Wqk^T @ qq -> fp8 DoubleRow layout [128, 2(g), 2(t), 196]."""
                k8 = qk8p.tile([128, 2, 2, HWP], FP8, tag="qk8")
                for g in range(2):
                    pk = ps_sim.tile([128, 392], F32, tag="ps_sim")
                    for t in range(2):
                        cc = 2 * g + t
                        nc.tensor.matmul(
                            pk[:, t * HW : (t + 1) * HW],
                            lhsT=wqk_sb[:, cc * 128 : (cc + 1) * 128],
                            rhs=qq_bf[:, b * HW : (b + 1) * HW],
                            start=True,
                            stop=True,
                        )
                    nc.vector.tensor_copy(
                        k8[:, g, :, 0:HW],
                        pk.rearrange("p (t hw) -> p t hw", t=2),
                    )
                qk8[b] = k8

            def sim_pair_gen(b):
                """Yield once per sim chunk pair: 4 matmuls + 1 exp on ACT."""
                e = etp.tile([128, NCH * HW], BF16, tag="et")
                nc.gpsimd.memset(e[:, 38 * HW :], 0.0)
                et[b] = e
                for jp in range(NPAIR):
                    chunks = (2 * jp, 2 * jp + 1) if jp < NPAIR - 1 else (38,)
                    ps = ps_sim.tile([128, 392], F32, tag="ps_sim")
                    for ci, j in enumerate(chunks):
                        cw = min(128, NIJ - j * 128)
                        for g in range(2):
                            nc.tensor.matmul(
                                ps[:cw, ci * HW : (ci + 1) * HW],
                                lhsT=s8[(b, g)][:, :, j * 128 : j * 128 + cw],
                                rhs=qk8[b][:, g, :, 0:HW],
                                start=(g == 0),
                                stop=(g == 1),
                                perf_mode=DR,
                            )
                    if jp < NPAIR - 1:
                        nc.scalar.activation(
                            out=e[:, 2 * jp * HW : (2 * jp + 2) * HW],
                            in_=ps,
                            func=EXP,
                        )
                    else:
                        nc.scalar.activation(
                            out=e[:36, 38 * HW :], in_=ps[:36, 0:HW], func=EXP
                        )
                    yield

            def vt_alloc(b):
                vt = vtp.tile([128, NCH * 132], BF16, tag="vt1")
                nc.gpsimd.memset(
                    vt.rearrange("p (j c) -> p j c", j=NCH)[:, :, 128:132], VSCALE
                )
                vt1[b] = vt

            def vt_gen(b, j_lo, j_hi, copy_eng0):
                """Yield per V^T chunk: 2 DoubleRow matmuls into a [128,512]
                PSUM quad tile; one wide copy per quad."""
                vtr = vt1[b].rearrange("p (j c) -> p j c", j=NCH)
                ncopy = copy_eng0
                for j0 in range(j_lo, j_hi, 4):
                    jn = min(4, j_hi - j0)
                    pq = ps_vt.tile([128, 512], F32, tag="ps_vt")
                    for ji in range(jn):
                        j = j0 + ji
                        cw = min(128, NIJ - j * 128)
                        for g in range(2):
                            nc.tensor.matmul(
                                pq[:cw, ji * 128 : (ji + 1) * 128],
                                lhsT=s8[(b, g)][:, :, j * 128 : j * 128 + cw],
                                rhs=wv8_sb[:, g],
                                start=(g == 0),
                                stop=(g == 1),
                                perf_mode=DR,
                            )
                        yield
                    nc.vector.tensor_copy(
                        vtr[:, j0 : j0 + jn, 0:128],
                        pq[:, 0 : jn * 128].rearrange("p (j c) -> p j c", j=jn),
                    )
                    ncopy += 1

            def pv_half_gen(b, h):
                """Yield per PV matmul (one et chunk each)."""
                vtr = vt1[b].rearrange("p (j c) -> p j c", j=NCH)
                hww = 128 if h == 0 else HW - 128
                pc = ps_ctx.tile([128, 132], F32, tag="ps_ctx")
                for j in range(NCH):
                    nc.tensor.matmul(
                        pc[:hww, :],
                        lhsT=et[b][:, j * HW + h * 128 : j * HW + h * 128 + hww],
                        rhs=vtr[:, j],
                        start=(j == 0),
                        stop=(j == NCH - 1),
                    )
                    yield
                # ctx epilogue avoids DVE entirely: late-run DVE queues behind
                # the tile framework's semaphore range-clear barriers.
                r = small.tile([128, 1], F32, tag="recip")
                nc.vector.reciprocal(r[:hww], pc[:hww, 128:129])
                d = small.tile([128, 128], F32, tag="diff")
                nc.vector.scalar_tensor_tensor(
                    d[:hww, :],
                    pc[:hww, 0:128],
                    r[:hww],
                    qvT[(b, h)][:hww, :],
                    op0=MULT,
                    op1=SUBTRACT,
                )
                d2 = small.tile([128, 128], F32, tag="d2")
                nc.vector.scalar_tensor_tensor(
                    d2[:hww, :],
                    d[:hww, :],
                    1.0,
                    d[:hww, :],
                    op0=MULT,
                    op1=MULT,
                    accum_out=partials[:hww, 2 * b + h : 2 * b + h + 1],
                )

            def drain(gen, n=None):
                if gen is None:
                    return None
                try:
                    if n is None:
                        while True:
                            next(gen)
                    else:
                        for _ in range(n):
                            next(gen)
                except StopIteration:
                    return None
                return gen

            # ---- schedule ----
            # PV(b) runs in-iteration, lagging its own exp stream by 2 slots;
            # its last 3 matmuls + ctx epilogue carry into iteration b+1.
            partials = const.tile([128, 2 * B_PER_CORE], F32, tag="partials")
            nc.vector.memset(partials, 0.0)
            q_proj(0, qq_bf, nc.vector)
            qk_prep(0)
            for i, (bq, hq) in enumerate(
                (b, h) for b in range(B_PER_CORE) for h in range(2)
            ):
                qvT_prep(bq, hq)
            vt_alloc(0)
            drain(vt_gen(0, 0, 19, 0))   # chunks covered by first two s8 quarters
            vt0_rest = vt_gen(0, 19, NCH, 1)
            carry = []

            for b in range(B_PER_CORE):
                simg = sim_pair_gen(b)
                if b + 1 < B_PER_CORE:
                    vt_alloc(b + 1)
                    vtg = vt_gen(b + 1, 0, NCH, 0)
                else:
                    vtg = None
                pvg = [pv_half_gen(b, h) for h in range(2)]
                for jp in range(NPAIR):
                    simg = drain(simg, 1)
                    if b == 0:
                        vt0_rest = drain(vt0_rest, 2)
                    if jp < 2 and carry:
                        carry = [drain(g, 2) for g in carry if g is not None]
                    if b > 0 or jp >= 2:
                        vtg = drain(vtg, 3 if b == 0 else 2)
                    if jp >= 1:
                        pvg = [drain(g, 2) for g in pvg]
                    if jp == 0 and b + 2 < B_PER_CORE:
                        s8_dma(b + 2, 2)
                    if jp == 8 and b + 1 < B_PER_CORE:
                        qk_prep(b + 1)
                drain(simg)
                drain(vtg)
                if b == 0:
                    drain(vt0_rest)
                for g in carry:
                    drain(g)
                carry = [g for g in pvg if g is not None]
            for g in carry:
                drain(g)

            # ---- final reduction to scalar ----
            ob = small.tile([1, 1], F32, tag="ob")
            nc.gpsimd.tensor_reduce(
                ob, partials, axis=mybir.AxisListType.XYZWC,
                op=mybir.AluOpType.add,
            )
            nc.sync.dma_start(out=out_d, in_=ob)

    nc.compile()
    return nc


_NC = None


def _prep_core(q, s, b0):
    """Host-side per-core input layouts (cast + transpose only)."""
    F8NP = ml_dtypes.float8_e4m3
    BFNP = ml_dtypes.bfloat16
    sb = s[b0 : b0 + B_PER_CORE]                      # [4, 25, 512, 196]
    s_c = sb.transpose(0, 2, 1, 3).reshape(B_PER_CORE, 2, 2, 128, NIJ)
    s8 = np.ascontiguousarray(s_c.transpose(0, 1, 3, 2, 4)).astype(F8NP)
    qb = q[b0 : b0 + B_PER_CORE]                      # [4, 512, 196]
    qbf = np.ascontiguousarray(
        qb.reshape(B_PER_CORE, 4, 128, HW).transpose(2, 1, 0, 3).reshape(
            128, 4, B_PER_CORE * HW
        )
    ).astype(BFNP)
    return {"s8": s8, "qbf": qbf}


def kernel(query_repr, supports_repr, W_qk, W_v):
    global _NC
    F8NP = ml_dtypes.float8_e4m3
    BFNP = ml_dtypes.bfloat16

    q = np.asarray(query_repr, dtype=np.float32).reshape(32, C, HW)
    s = np.asarray(supports_repr, dtype=np.float32).reshape(32, N_SUP, C, HW)
    wqk = np.asarray(W_qk, dtype=np.float32)
    wv = np.asarray(W_v, dtype=np.float32)

    wqk_bf = wqk.astype(BFNP)
    # wT[p, 0:4, dk] = Wqk^T chunks, wT[p, 4:8, dk] = Wv^T chunks
    wT = np.concatenate(
        [wqk.T.reshape(4, 128, DK), wv.T.reshape(4, 128, DK)], axis=0
    ).transpose(1, 0, 2)
    wT = np.ascontiguousarray(wT).astype(BFNP)
    wv8 = np.ascontiguousarray(
        (VSCALE * wv).T.reshape(2, 2, 128, DK).transpose(2, 0, 1, 3)
    ).astype(F8NP)

    if _NC is None:
        _NC = build_bass()

    in_maps = []
    for core in range(8):
        m = _prep_core(q, s, core * B_PER_CORE)
        m.update({"wqk": wqk_bf, "wT": wT, "wv8": wv8})
        in_maps.append(m)
    res = run_bass_kernel_spmd(
        _NC, in_maps, core_ids=list(range(8)),
        trace=bool(int(os.environ.get("KTRACE", "0"))),
    )
    total = sum(float(r["out"][0, 0]) for r in res.results) / float(HW)
    kernel._last_results = res
    return np.asarray(total, dtype=np.float32)


# revision 4
# speedup vs baseline: 3.8289x; 1.0018x over previous
"""CrossTransformer kernel v4 for Trainium2 — fp8 sim-direct, direct-V^T.

Per batch b (B=32 -> 4/core, N=25, C=512, H=W=14, DK=DV=128):
  qq = Wqk @ Q   (bf16)        qv = Wv @ Q  (bf16 -> f32)
  qk = Wqk^T @ qq  -> fp8 e4m3 DoubleRow layout [g][p][t][hw], c = g*256+t*128+p
  sim[nij,hw] = S^T @ qk       (fp8 DoubleRow; S is never projected to K)
  E = exp(sim) bf16            (ACT, 392-wide PSUM chunk pairs)
  V^T[nij,dv] = S^T @ (16*Wv)^T  (fp8 DoubleRow, direct transposed layout)
  ctx_raw[hw,132] = sum_j E_j^T @ [V^T_j | 16]   (ones=16 cancels the Wv scale)
  partial += sum((qv^T - num*recip(den))^2)

Schedule: iteration b emits sim(b) interleaved with V^T(b+1), qk(b+1) and
PV(b) (both halves, 2 matmuls per slot, lagging the exp stream by 1 slot;
tails carry into iteration b+1) so the PE never head-of-line blocks on exp
or PSUM drains; the ACT exp stream (~10.4us/batch) is the pacing engine.
GPSIMD touches only SBUF (PSUM access is rejected by the BIR verifier) and
DoubleRow operand k-tile strides are padded to 16B alignment (4912/208).
"""

import os
import sys

sys.path.insert(0, "/opt/trn_rl_repo")

import numpy as np
import ml_dtypes

import concourse.bass as bass
import concourse.bacc as bacc
import concourse.mybir as mybir
import concourse.tile as tile
from concourse.bass_utils import run_bass_kernel_spmd
from concourse.masks import make_identity

F32 = mybir.dt.float32
BF16 = mybir.dt.bfloat16
FP8 = mybir.dt.float8e4

B_PER_CORE = 4
N_SUP = 25
C = 512
HW = 196
NIJ = N_SUP * HW          # 4900
DK = 128
NCH = 39                  # nij chunks of <=128 (last = 36 rows)
NIJP = 4912               # s8 SBUF row pitch: 16B-aligned for DoubleRow
HWP = 208                 # qk8 row pitch: 16B-aligned for DoubleRow
NPAIR = 20                # sim chunk pairs (last = chunk 38 alone)
VSCALE = 16.0             # host scales Wv by 16; ones column = 16 cancels it

DR = mybir.MatmulPerfMode.DoubleRow
EXP = mybir.ActivationFunctionType.Exp
RCP = mybir.ActivationFunctionType.Reciprocal
MULT = mybir.AluOpType.mult
SUBTRACT = mybir.AluOpType.subtract
ADD = mybir.AluOpType.add


def build_bass():
    nc = bacc.Bacc(
        "TRN2", target_bir_lowering=False, debug=False, enable_asserts=False
    )
    s8_d = nc.dram_tensor(
        "s8", [B_PER_CORE, 2, 128, 2, NIJ], FP8, kind="ExternalInput"
    ).ap()
    # packed: [p, cc0..3]=WqkT chunks, [p, 4..7]=WvT chunks
    wT_d = nc.dram_tensor("wT", [128, 8, DK], BF16, kind="ExternalInput").ap()
    q_d = nc.dram_tensor(
        "qbf", [128, 4, B_PER_CORE * HW], BF16, kind="ExternalInput"
    ).ap()
    wqk_d = nc.dram_tensor("wqk", [DK, C], BF16, kind="ExternalInput").ap()
    wv8_d = nc.dram_tensor("wv8", [128, 2, 2, DK], FP8, kind="ExternalInput").ap()
    out_d = nc.dram_tensor(
        "out", [128, 2 * B_PER_CORE], F32, kind="ExternalOutput"
    ).ap()

    with tile.TileContext(nc) as tc:
        with (
            tc.tile_pool(name="const", bufs=1) as const,
            tc.tile_pool(name="s8p", bufs=8) as s8p,
            tc.tile_pool(name="etp", bufs=2) as etp,
            tc.tile_pool(name="vtp", bufs=3) as vtp,
            tc.tile_pool(name="qk8p", bufs=4) as qk8p,
            tc.tile_pool(name="small", bufs=8) as small,
            tc.tile_pool(name="ps_sim", bufs=4, space="PSUM") as ps_sim,
            tc.tile_pool(name="ps_vt", bufs=2, space="PSUM") as ps_vt,
            tc.tile_pool(name="ps_ctx", bufs=2, space="PSUM") as ps_ctx,
        ):
            # ---- input DMAs, ordered for fastest time-to-first-exp ----
            wT_sb = const.tile([128, 8, DK], BF16, tag="wT_sb")
            nc.sync.dma_start(out=wT_sb, in_=wT_d)
            q_sb = const.tile([128, 4, B_PER_CORE * HW], BF16, tag="q_sb")
            nc.sync.dma_start(out=q_sb[:, 0:2], in_=q_d[:, 0:2])
            nc.sync.dma_start(out=q_sb[:, 2:4], in_=q_d[:, 2:4])
            wqk_sb = const.tile([128, C], BF16, tag="wqk_sb")
            nc.sync.dma_start(out=wqk_sb, in_=wqk_d)

            s8 = {}

            def s8_alloc(b):
                for g in range(2):
                    s8t = s8p.tile([128, 2, NIJP], FP8, tag="s8")
                    s8[(b, g)] = s8t

            def s8_piece(b, o, ln):
                for g in range(2):
                    nc.sync.dma_start(
                        out=s8[(b, g)][:, :, o : o + ln],
                        in_=s8_d[b, g][:, :, o : o + ln],
                    )

            def s8_dma(b, pieces):
                s8_alloc(b)
                w = NIJ // pieces
                for i in range(pieces):
                    o = i * w
                    s8_piece(b, o, w if i < pieces - 1 else NIJ - o)

            # prologue pieces ordered so sim(0)/vt(0)/vt(1) prerequisites
            # land just in time (HWDGE serializes at 625ns per DMA)
            s8_alloc(0)
            s8_alloc(1)
            s8_piece(0, 0, 1225)
            wv8_sb = const.tile([128, 2, 2, DK], FP8, tag="wv8_sb")
            nc.sync.dma_start(out=wv8_sb, in_=wv8_d)
            s8_piece(1, 0, 2450)
            s8_piece(0, 1225, 1225)
            s8_piece(0, 2450, 2450)
            s8_piece(1, 2450, 2450)

            # PE p-state warmup: wide matmuls on a zero tile keep the PE
            # continuously busy through the DMA wait so the real prologue
            # runs at full clock (ap 512 > write latency -> no WAW stall)
            warm_src = const.tile([128, 512], BF16, tag="warm_src")
            nc.gpsimd.memset(warm_src, 0.0)
            for i in range(12):
                pw = ps_vt.tile([128, 512], F32, tag="ps_vt")
                nc.tensor.matmul(
                    pw,
                    lhsT=warm_src[:, 0:128],
                    rhs=warm_src,
                    start=True,
                    stop=True,
                )


            # ---- qq projection (all 4 batches at once) ----
            qq_bf = const.tile([128, B_PER_CORE * HW], BF16, tag="qq_bf")

            def q_proj(wo, dst, eng):
                for half in range(2):
                    hw0 = half * 392
                    pq = ps_sim.tile([128, 392], F32, tag="ps_sim")
                    for cc in range(4):
                        nc.tensor.matmul(
                            pq,
                            lhsT=wT_sb[:, wo + cc],
                            rhs=q_sb[:, cc, hw0 : hw0 + 392],
                            start=(cc == 0),
                            stop=(cc == 3),
                        )
                    eng.tensor_copy(dst[:, hw0 : hw0 + 392], pq)

            qvT = {}

            def qvT_prep(b, h):
                """qv^T[hw, dk] computed directly: lhsT = Q chunk, rhs = Wv^T."""
                hww = 128 if h == 0 else HW - 128
                o = b * HW + h * 128
                pt = ps_sim.tile([128, 392], F32, tag="ps_sim")
                for cc in range(4):
                    nc.tensor.matmul(
                        pt[:hww, 0:128],
                        lhsT=q_sb[:, cc, o : o + hww],
                        rhs=wT_sb[:, 4 + cc],
                        start=(cc == 0),
                        stop=(cc == 3),
                    )
                qt = const.tile([128, 128], F32, tag=f"qvT{b}_{h}")
                nc.vector.tensor_copy(qt[:hww, :], pt[:hww, 0:128])
                qvT[(b, h)] = qt


            # ---- per-batch stage generators (interleavable) ----
            et = {}
            vt1 = {}
            qk8 = {}

            def qk_prep(b):
                """qk = Wqk^T @ qq -> fp8 DoubleRow layout [128, 2(g), 2(t), 196]."""
                k8 = qk8p.tile([128, 2, 2, HWP], FP8, tag="qk8")
                for g in range(2):
                    pk = ps_sim.tile([128, 392], F32, tag="ps_sim")
                    for t in range(2):
                        cc = 2 * g + t
                        nc.tensor.matmul(
                            pk[:, t * HW : (t + 1) * HW],
                            lhsT=wqk_sb[:, cc * 128 : (cc + 1) * 128],
                            rhs=qq_bf[:, b * HW : (b + 1) * HW],
                            start=True,
                            stop=True,
                        )
                    nc.vector.tensor_copy(
                        k8[:, g, :, 0:HW],
                        pk.rearrange("p (t hw) -> p t hw", t=2),
                    )
                qk8[b] = k8

            def sim_pair_gen(b):
                """Yield once per sim chunk pair: 4 matmuls + 1 exp on ACT."""
                e = etp.tile([128, NCH * HW], BF16, tag="et")
                nc.gpsimd.memset(e[:, 38 * HW :], 0.0)
                et[b] = e
                for jp in range(NPAIR):
                    chunks = (2 * jp, 2 * jp + 1) if jp < NPAIR - 1 else (38,)
                    ps = ps_sim.tile([128, 392], F32, tag="ps_sim")
                    for ci, j in enumerate(chunks):
                        cw = min(128, NIJ - j * 128)
                        for g in range(2):
                            nc.tensor.matmul(
                                ps[:cw, ci * HW : (ci + 1) * HW],
                                lhsT=s8[(b, g)][:, :, j * 128 : j * 128 + cw],
                                rhs=qk8[b][:, g, :, 0:HW],
                                start=(g == 0),
                                stop=(g == 1),
                                perf_mode=DR,
                            )
                    if jp < NPAIR - 1:
                        nc.scalar.activation(
                            out=e[:, 2 * jp * HW : (2 * jp + 2) * HW],
                            in_=ps,
                            func=EXP,
                        )
                    else:
                        nc.scalar.activation(
                            out=e[:36, 38 * HW :], in_=ps[:36, 0:HW], func=EXP
                        )
                    yield

            def vt_alloc(b):
                vt = vtp.tile([128, NCH * 132], BF16, tag="vt1")
                nc.gpsimd.memset(
                    vt.rearrange("p (j c) -> p j c", j=NCH)[:, :, 128:132], VSCALE
                )
                vt1[b] = vt

            def vt_gen(b, j_lo, j_hi, copy_eng0):
                """Yield per V^T chunk: 2 DoubleRow matmuls into a [128,512]
                PSUM quad tile; one wide copy per quad."""
                vtr = vt1[b].rearrange("p (j c) -> p j c", j=NCH)
                ncopy = copy_eng0
                for j0 in range(j_lo, j_hi, 4):
                    jn = min(4, j_hi - j0)
                    pq = ps_vt.tile([128, 512], F32, tag="ps_vt")
                    for ji in range(jn):
                        j = j0 + ji
                        cw = min(128, NIJ - j * 128)
                        for g in range(2):
                            nc.tensor.matmul(
                                pq[:cw, ji * 128 : (ji + 1) * 128],
                                lhsT=s8[(b, g)][:, :, j * 128 : j * 128 + cw],
                                rhs=wv8_sb[:, g],
                                start=(g == 0),
                                stop=(g == 1),
                                perf_mode=DR,
                            )
                        yield
                    nc.vector.tensor_copy(
                        vtr[:, j0 : j0 + jn, 0:128],
                        pq[:, 0 : jn * 128].rearrange("p (j c) -> p j c", j=jn),
                    )
                    ncopy += 1

            def pv_half_gen(b, h):
                """Yield per PV matmul (one et chunk each)."""
                vtr = vt1[b].rearrange("p (j c) -> p j c", j=NCH)
                hww = 128 if h == 0 else HW - 128
                pc = ps_ctx.tile([128, 132], F32, tag="ps_ctx")
                for j in range(NCH):
                    nc.tensor.matmul(
                        pc[:hww, :],
                        lhsT=et[b][:, j * HW + h * 128 : j * HW + h * 128 + hww],
                        rhs=vtr[:, j],
                        start=(j == 0),
                        stop=(j == NCH - 1),
                    )
                    yield
                # ctx epilogue avoids DVE entirely: late-run DVE queues behind
                # the tile framework's semaphore range-clear barriers.
                r = small.tile([128, 1], F32, tag="recip")
                nc.vector.reciprocal(r[:hww], pc[:hww, 128:129])
                d = small.tile([128, 128], F32, tag="diff")
                nc.vector.scalar_tensor_tensor(
                    d[:hww, :],
                    pc[:hww, 0:128],
                    r[:hww],
                    qvT[(b, h)][:hww, :],
                    op0=MULT,
                    op1=SUBTRACT,
                )
                d2 = small.tile([128, 128], F32, tag="d2")
                nc.vector.scalar_tensor_tensor(
                    d2[:hww, :],
                    d[:hww, :],
                    1.0,
                    d[:hww, :],
                    op0=MULT,
                    op1=MULT,
                    accum_out=partials[:hww, 2 * b + h : 2 * b + h + 1],
                )

            def drain(gen, n=None):
                if gen is None:
                    return None
                try:
                    if n is None:
                        while True:
                            next(gen)
                    else:
                        for _ in range(n):
                            next(gen)
                except StopIteration:
                    return None
                return gen

            # ---- schedule ----
            # PV(b) runs in-iteration, lagging its own exp stream by 2 slots;
            # its last 3 matmuls + ctx epilogue carry into iteration b+1.
            partials = const.tile([128, 2 * B_PER_CORE], F32, tag="partials")
            nc.vector.memset(partials, 0.0)
            q_proj(0, qq_bf, nc.vector)
            qk_prep(0)
            for i, (bq, hq) in enumerate(
                (b, h) for b in range(B_PER_CORE) for h in range(2)
            ):
                qvT_prep(bq, hq)
            vt_alloc(0)
            drain(vt_gen(0, 0, 19, 0))   # chunks covered by first two s8 quarters
            vt0_rest = vt_gen(0, 19, NCH, 1)
            carry = []

            for b in range(B_PER_CORE):
                simg = sim_pair_gen(b)
                if b + 1 < B_PER_CORE:
                    vt_alloc(b + 1)
                    vtg = vt_gen(b + 1, 0, NCH, 0)
                else:
                    vtg = None
                pvg = [pv_half_gen(b, h) for h in range(2)]
                for jp in range(NPAIR):
                    simg = drain(simg, 1)
                    if b == 0:
                        vt0_rest = drain(vt0_rest, 2)
                    if jp < 2 and carry:
                        carry = [drain(g, 2) for g in carry if g is not None]
                    if b > 0 or jp >= 2:
                        vtg = drain(vtg, 3 if b == 0 else 2)
                    if jp >= 1:
                        pvg = [drain(g, 2) for g in pvg]
                    if jp == 0 and b + 2 < B_PER_CORE:
                        s8_dma(b + 2, 2)
                    if jp == 8 and b + 1 < B_PER_CORE:
                        qk_prep(b + 1)
                drain(simg)
                drain(vtg)
                if b == 0:
                    drain(vt0_rest)
                for g in carry:
                    drain(g)
                carry = [g for g in pvg if g is not None]
            for g in carry:
                drain(g)

            # final reduction happens on the host: DMA the partials matrix
            nc.sync.dma_start(out=out_d, in_=partials)

    nc.compile()
    return nc


_NC = None


def _prep_core(q, s, b0):
    """Host-side per-core input layouts (cast + transpose only)."""
    F8NP = ml_dtypes.float8_e4m3
    BFNP = ml_dtypes.bfloat16
    sb = s[b0 : b0 + B_PER_CORE]                      # [4, 25, 512, 196]
    s_c = sb.transpose(0, 2, 1, 3).reshape(B_PER_CORE, 2, 2, 128, NIJ)
    s8 = np.ascontiguousarray(s_c.transpose(0, 1, 3, 2, 4)).astype(F8NP)
    qb = q[b0 : b0 + B_PER_CORE]                      # [4, 512, 196]
    qbf = np.ascontiguousarray(
        qb.reshape(B_PER_CORE, 4, 128, HW).transpose(2, 1, 0, 3).reshape(
            128, 4, B_PER_CORE * HW
        )
    ).astype(BFNP)
    return {"s8": s8, "qbf": qbf}


def kernel(query_repr, supports_repr, W_qk, W_v):
    global _NC
    F8NP = ml_dtypes.float8_e4m3
    BFNP = ml_dtypes.bfloat16

    q = np.asarray(query_repr, dtype=np.float32).reshape(32, C, HW)
    s = np.asarray(supports_repr, dtype=np.float32).reshape(32, N_SUP, C, HW)
    wqk = np.asarray(W_qk, dtype=np.float32)
    wv = np.asarray(W_v, dtype=np.float32)

    wqk_bf = wqk.astype(BFNP)
    # wT[p, 0:4, dk] = Wqk^T chunks, wT[p, 4:8, dk] = Wv^T chunks
    wT = np.concatenate(
        [wqk.T.reshape(4, 128, DK), wv.T.reshape(4, 128, DK)], axis=0
    ).transpose(1, 0, 2)
    wT = np.ascontiguousarray(wT).astype(BFNP)
    wv8 = np.ascontiguousarray(
        (VSCALE * wv).T.reshape(2, 2, 128, DK).transpose(2, 0, 1, 3)
    ).astype(F8NP)

    if _NC is None:
        _NC = build_bass()

    in_maps = []
    for core in range(8):
        m = _prep_core(q, s, core * B_PER_CORE)
        m.update({"wqk": wqk_bf, "wT": wT, "wv8": wv8})
        in_maps.append(m)
    res = run_bass_kernel_spmd(
        _NC, in_maps, core_ids=list(range(8)),
        trace=bool(int(os.environ.get("KTRACE", "0"))),
    )
    total = sum(float(r["out"].astype(np.float64).sum()) for r in res.results)
    total = total / float(HW)
    kernel._last_results = res
    return np.asarray(total, dtype=np.float32)
